# revision 1
# baseline (speedup 1.0000x reference)
"""Masked multi-head attention (B=32, Lq=Lk=512, H=20, D=20) on 8 TRN2 NeuronCores.

Strategy:
  - Data-parallel over batch: 32 batches -> 8 cores x 4 "slots" (SPMD: one NEFF).
  - Host bakes per-slot static shapes (nq = padded Q_len, nkc = kv chunks from
    V_len) and bin-packs batches into slot groups to minimize padded work.
  - Host pre-transposes sequences to [21, L] (20 features + ones row).  The
    ones row realizes: exact linear bias, zeroing of masked kv positions
    (mask folded into V/K inputs), and a free softmax-denominator column in
    the projected V tile.
  - Device per (slot, head-group of 4 heads at 32-partition offsets):
      proj Q/K/V (PE, contraction 21)
      S^T = K_h @ Q_h^T   row-tiled 4 heads concurrently  (PSUM)
      P^T = exp(S^T / sqrt(D))  one ACTIVATE per pack     (SBUF)
      O^T(+sums row) = [V_h|mask]^T @ P^T  col-tiled, accumulated over kv chunks
      PE transpose -> [q, .] layout; DVE reciprocal + broadcast multiply
      assemble [128, 400] and DMA to DRAM.
  - Host scatters per-slot outputs into the final [32, 512, 400] (rows beyond
    Q_len stay zero, which implements the multiplicative q mask exactly).
"""

import math
import random

import numpy as np

import concourse.bacc as bacc
import concourse.bass as bass
import concourse.tile as tile
from concourse import mybir
from concourse.bass_utils import run_bass_kernel_spmd

B, LQ, LK = 32, 512, 512
H, D = 20, 20
OUT_DIM = H * D  # 400
N_CORES = 8
N_SLOTS = B // N_CORES  # 4
QCH = 128
KCH = 128
NG = 5  # head groups
HPG = 4  # heads per group (at partition offsets 0/32/64/96)
VW = H * 21 + 12  # 432 (even, for fp32r): per-head 20 dims + 1 ones col,
                  # padded so a 32-wide lhsT slice exists for every head
SCALE = 1.0 / math.sqrt(D)
# Constant shift inside exp: P = exp(s/sqrt(D) - ESHIFT).  Softmax is
# shift-invariant (sums scale by e^-ESHIFT), and the shift keeps P below
# fp16 max (65504) for scores up to ~17 sigma.  Zero-flush of the tiniest
# weights (P < 6e-8) is harmless: they are >= e^9 below their column max.
ESHIFT = 6.0

F32 = mybir.dt.float32

# Perf knobs
USE_F32R = True  # bitcast matmul operands to float32r (fast fp32 path)
TRACE = False  # set True to capture NTFF profile (slower)
LAST_RESULT = None  # BassKernelResults of the last run (for test harness)


# ----------------------------------------------------------------- planning

def _plan(q_len, v_len):
    """Group 32 batches into N_SLOTS groups of N_CORES, minimizing baked cost.

    Returns list of (nq, nkc, batches[8]) sorted big->small."""
    nqc = [max(1, math.ceil(min(int(q), LQ) / QCH)) for q in q_len]
    kv_eff = [LK if int(v) <= 0 else min(int(v), LK) for v in v_len]
    nkc = [math.ceil(k / KCH) for k in kv_eff]
    cost = [a * b for a, b in zip(nqc, nkc)]
    order = sorted(range(B), key=lambda b: -cost[b])

    def baked(gs):
        t = 0
        for g in gs:
            if g:
                t += max(nqc[b] for b in g) * max(nkc[b] for b in g)
        return t

    groups = [[] for _ in range(N_SLOTS)]
    for b in order:
        best, bestc = None, None
        for gi in range(N_SLOTS):
            if len(groups[gi]) >= N_CORES:
                continue
            groups[gi].append(b)
            c = baked(groups)
            groups[gi].pop()
            if bestc is None or c < bestc:
                best, bestc = gi, c
        groups[best].append(b)
    rng = random.Random(0)
    cur = baked(groups)
    for _ in range(6000):
        g1, g2 = rng.randrange(N_SLOTS), rng.randrange(N_SLOTS)
        if g1 == g2:
            continue
        i1, i2 = rng.randrange(N_CORES), rng.randrange(N_CORES)
        groups[g1][i1], groups[g2][i2] = groups[g2][i2], groups[g1][i1]
        c = baked(groups)
        if c <= cur:
            cur = c
        else:
            groups[g1][i1], groups[g2][i2] = groups[g2][i2], groups[g1][i1]
    slots = []
    for g in groups:
        snq = max(nqc[b] for b in g) * QCH
        snkc = max(nkc[b] for b in g)
        slots.append((snq, snkc, list(g)))
    slots.sort(key=lambda s: -(s[0] * s[1]))
    return slots


# ------------------------------------------------------------ host packing

def _pack_qk_weights(W, bias):
    """[400, 20] linear weight -> [21, NG*128] lhsT layout (head 4g+j at
    columns 128g+32j .. +20; row 20 = bias)."""
    t = np.zeros((D + 1, NG * 128), np.float32)
    for h in range(H):
        g, j = divmod(h, HPG)
        c = g * 128 + 32 * j
        t[:D, c:c + D] = W[h * D:(h + 1) * D, :].T
        t[D, c:c + D] = bias[h * D:(h + 1) * D]
    return t


def _pack_v_weights(W, bias):
    """[400, 20] -> [21, 420] rhs layout: head h at cols 21h..21h+19,
    ones-generator col at 21h+20."""
    t = np.zeros((D + 1, VW), np.float32)
    for h in range(H):
        c = 21 * h
        t[:D, c:c + D] = W[h * D:(h + 1) * D, :].T
        t[D, c:c + D] = bias[h * D:(h + 1) * D]
        t[D, c + D] = 1.0
    return t


def _prep_qt(qs, nq):
    t = np.zeros((D + 1, nq), np.float32)
    n = min(nq, LQ)
    t[:D, :n] = qs[:n].T
    t[D, :n] = 1.0
    return t


def _prep_kvt(ks, vlen, nkv):
    """K/V sequence transposed with ones row; columns >= V_len zeroed
    (vlen==0 means "uniform -1e12 shift" in the reference == full attention)."""
    t = np.zeros((D + 1, nkv), np.float32)
    n = min(nkv, LK) if int(vlen) <= 0 else min(nkv, int(vlen))
    t[:D, :n] = ks[:n].T
    t[D, :n] = 1.0
    return t


# ------------------------------------------------------------ device build

def _emit(tc, nc, dr, slots):
    # fp32r matmul operands must come from instructions that round to fp32r;
    # DMA can't, so DMA'd tensors get one DVE rounding copy each.
    DT = mybir.dt.float32r if USE_F32R else F32
    with (
        tc.tile_pool(name="wpool", bufs=1) as wpool,
        tc.tile_pool(name="seqin", bufs=2) as seqp,
        tc.tile_pool(name="sbq", bufs=3) as sbqp,
        tc.tile_pool(name="sbk", bufs=3) as sbkp,
        tc.tile_pool(name="sbv", bufs=6) as sbvp,
        tc.tile_pool(name="sbp", bufs=4) as sbpp,
        tc.tile_pool(name="sbo", bufs=2) as sbop,
        tc.tile_pool(name="sbr", bufs=4) as sbrp,
        tc.tile_pool(name="asm", bufs=6) as asmp,
        tc.tile_pool(name="ppj", bufs=1, space="PSUM") as ppj,
        tc.tile_pool(name="pss", bufs=2, space="PSUM") as pss,
        tc.tile_pool(name="pso", bufs=2, space="PSUM") as pso,
        tc.tile_pool(name="pst", bufs=1, space="PSUM") as pst,
    ):
        def load_rounded(name, shape, pool, tag):
            raw = pool.tile(shape, F32, tag=tag + "_raw", name=name + "_raw")
            nc.sync.dma_start(raw[:], dr[name])
            if not USE_F32R:
                return raw
            t = pool.tile(shape, DT, tag=tag, name=name + "_r")
            nc.vector.tensor_copy(t[:], raw[:])
            return t

        wq = load_rounded("wq", [D + 1, NG * 128], wpool, "wq")
        wk = load_rounded("wk", [D + 1, NG * 128], wpool, "wk")
        wv = load_rounded("wv", [D + 1, VW], wpool, "wv")
        ident = load_rounded("ident", [128, 128], wpool, "ident")
        eshift = wpool.tile([128, 1], F32, tag="eshift")
        nc.vector.memset(eshift[:], -ESHIFT)

        for s, (nq, nkc, _g) in enumerate(slots):
            nkv = nkc * KCH
            nqc = nq // QCH
            # 2 heads per S^T psum tile; each head's [128, nq] slice padded to a
            # full 2KB bank so no two matmul outputs share a PSUM zero region.
            hp = 2

            qt = load_rounded(f"qt{s}", [D + 1, nq], seqp, "qt")
            kt = load_rounded(f"kt{s}", [D + 1, nkv], seqp, "kt")
            vt = load_rounded(f"vt{s}", [D + 1, nkv], seqp, "vt")

            # V projection: per kv chunk -> [128, 420] (incl. masked ones cols)
            sbV = []
            for kc in range(nkc):
                pv = ppj.tile([128, 512], F32, tag="ppj")
                nc.tensor.matmul(
                    pv[:, :VW], vt[:, kc * KCH:(kc + 1) * KCH], wv[:],
                    start=True, stop=True,
                )
                v = sbvp.tile([128, VW], mybir.dt.float16, tag="sbv")
                nc.vector.tensor_copy(v[:], pv[:, :VW])
                sbV.append(v)

            asms = [
                asmp.tile([128, OUT_DIM], F32, tag="asm", name=f"asm{s}_{qc}")
                for qc in range(nqc)
            ]

            for g in range(NG):
                pq = ppj.tile([128, 512], F32, tag="ppj")
                nc.tensor.matmul(
                    pq[:, :nq], wq[:, g * 128:(g + 1) * 128], qt[:],
                    start=True, stop=True,
                )
                q = sbqp.tile([128, nq], mybir.dt.float16, tag="sbq")
                nc.vector.tensor_copy(q[:], pq[:, :nq])

                pk = ppj.tile([128, 512], F32, tag="ppj")
                nc.tensor.matmul(
                    pk[:, :nkv], wk[:, g * 128:(g + 1) * 128], kt[:],
                    start=True, stop=True,
                )
                k = sbkp.tile([128, nkv], mybir.dt.float16, tag="sbk")
                nc.vector.tensor_copy(k[:], pk[:, :nkv])

                po = pso.tile([128, nq], F32, tag="pso")

                for kc in range(nkc):
                    # all 4 S^T matmuls back-to-back (distinct row groups ->
                    # they pipeline/overlap in the PE's 32x32 subarrays),
                    # then the exps, then the 4 O^T matmuls (distinct col
                    # groups).  Interleaving full-row-span work between
                    # row-tiled matmuls would serialize the subarrays.
                    packs = []
                    for jp in range(0, HPG, hp):
                        ps = pss.tile([128, hp, 512], F32, tag="pss",
                                      name=f"ps{s}_{g}_{kc}_{jp}")
                        for j in range(jp, jp + hp):
                            nc.tensor.matmul(
                                ps[:, j - jp, :nq],
                                k[32 * j:32 * j + D, kc * KCH:(kc + 1) * KCH],
                                q[32 * j:32 * j + D, :],
                                start=True, stop=True,
                                tile_position=(32 * j, 0),
                            )
                        packs.append(ps)
                    ptiles = []
                    for jp, ps in zip(range(0, HPG, hp), packs):
                        p = sbpp.tile([128, hp, 512], mybir.dt.float16,
                                      tag="sbp", name=f"p{s}_{g}_{kc}_{jp}")
                        nc.scalar.activation(
                            p[:, :, :nq], ps[:, :, :nq],
                            mybir.ActivationFunctionType.Exp,
                            bias=eshift[:], scale=SCALE,
                        )
                        ptiles.append(p)
                    for jp, p in zip(range(0, HPG, hp), ptiles):
                        for j in range(jp, jp + hp):
                            h = HPG * g + j
                            # col-tiled accumulation chains touch disjoint
                            # partition ranges (32j..32j+20) of one bank; the
                            # sim's zero-region check is bank-granular, so
                            # bypass it.
                            nc.tensor.matmul(
                                po[32 * j:32 * j + 32, :],
                                sbV[kc][:, 21 * h:21 * h + 32],
                                p[:, j - jp, :nq],
                                start=(kc == 0), stop=(kc == nkc - 1),
                                tile_position=(0, 32 * j),
                                skip_group_check=True,
                            )

                o = sbop.tile([128, nq], DT, tag="sbo")
                nc.vector.tensor_copy(o[:], po[:])
                for qc in range(nqc):
                    pt = pst.tile([128, 128], DT, tag="pst")
                    nc.tensor.transpose(pt[:], o[:, qc * QCH:(qc + 1) * QCH], ident[:])
                    # f32r bits are valid f32; read back as f32 for DVE ops
                    ptb = pt.bitcast(F32).rearrange("p (j c) -> p j c", j=HPG)
                    r = sbrp.tile([128, HPG], F32, tag="sbr")
                    nc.vector.reciprocal(r[:], ptb[:, :, D])
                    nc.vector.tensor_mul(
                        asms[qc][:, g * 80:(g + 1) * 80]
                            .rearrange("p (j d) -> p j d", j=HPG),
                        ptb[:, :, 0:D],
                        r.unsqueeze(2).broadcast_to([128, HPG, D]),
                    )

            for qc in range(nqc):
                nc.sync.dma_start(
                    dr[f"o{s}"][qc * QCH:(qc + 1) * QCH, :], asms[qc][:]
                )


def _build_nc(slots):
    nc = bacc.Bacc(
        "TRN2",
        target_bir_lowering=False,
        debug=False,
        enable_asserts=False,
        num_devices=N_CORES,
    )
    dr = {}
    for s, (nq, nkc, _grp) in enumerate(slots):
        nkv = nkc * KCH
        dr[f"qt{s}"] = nc.dram_tensor(f"qt{s}", [D + 1, nq], F32, kind="ExternalInput").ap()
        dr[f"kt{s}"] = nc.dram_tensor(f"kt{s}", [D + 1, nkv], F32, kind="ExternalInput").ap()
        dr[f"vt{s}"] = nc.dram_tensor(f"vt{s}", [D + 1, nkv], F32, kind="ExternalInput").ap()
        dr[f"o{s}"] = nc.dram_tensor(f"o{s}", [nq, OUT_DIM], F32, kind="ExternalOutput").ap()
    dr["wq"] = nc.dram_tensor("wq", [D + 1, NG * 128], F32, kind="ExternalInput").ap()
    dr["wk"] = nc.dram_tensor("wk", [D + 1, NG * 128], F32, kind="ExternalInput").ap()
    dr["wv"] = nc.dram_tensor("wv", [D + 1, VW], F32, kind="ExternalInput").ap()
    dr["ident"] = nc.dram_tensor("ident", [128, 128], F32, kind="ExternalInput").ap()

    with tile.TileContext(nc) as tc:
        _emit(tc, nc, dr, slots)
    nc.compile()
    return nc


# ----------------------------------------------------------------- driver

def kernel(**inputs):
    global LAST_RESULT
    Q_seq = np.ascontiguousarray(np.asarray(inputs["Q_seq"], dtype=np.float32))
    K_seq = np.ascontiguousarray(np.asarray(inputs["K_seq"], dtype=np.float32))
    V_seq = np.ascontiguousarray(np.asarray(inputs["V_seq"], dtype=np.float32))
    Q_len = np.asarray(inputs["Q_len"]).reshape(-1).astype(np.int64)
    V_len = np.asarray(inputs["V_len"]).reshape(-1).astype(np.int64)
    WQ_w = np.asarray(inputs["WQ_w"], dtype=np.float32)
    WQ_b = np.asarray(inputs["WQ_b"], dtype=np.float32)
    WK_w = np.asarray(inputs["WK_w"], dtype=np.float32)
    WK_b = np.asarray(inputs["WK_b"], dtype=np.float32)
    WV_w = np.asarray(inputs["WV_w"], dtype=np.float32)
    WV_b = np.asarray(inputs["WV_b"], dtype=np.float32)

    slots = _plan(Q_len, V_len)
    nc = _build_nc(slots)

    wq = _pack_qk_weights(WQ_w, WQ_b)
    wk = _pack_qk_weights(WK_w, WK_b)
    wv = _pack_v_weights(WV_w, WV_b)
    ident = np.eye(128, dtype=np.float32)

    in_maps = []
    for c in range(N_CORES):
        m = {"wq": wq, "wk": wk, "wv": wv, "ident": ident}
        for s, (nq, nkc, grp) in enumerate(slots):
            b = grp[c]
            nkv = nkc * KCH
            m[f"qt{s}"] = _prep_qt(Q_seq[b], nq)
            m[f"kt{s}"] = _prep_kvt(K_seq[b], V_len[b], nkv)
            m[f"vt{s}"] = _prep_kvt(V_seq[b], V_len[b], nkv)
        in_maps.append(m)

    res = run_bass_kernel_spmd(
        nc, in_maps, core_ids=list(range(N_CORES)), trace=TRACE
    )
    LAST_RESULT = res

    out = np.zeros((B, LQ, OUT_DIM), np.float32)
    for c in range(N_CORES):
        for s, (_nq, _nkc, grp) in enumerate(slots):
            b = grp[c]
            ql = int(Q_len[b])
            if ql > 0:
                out[b, :ql] = res.results[c][f"o{s}"][:ql]
    return out



# revision 7
# speedup vs baseline: 1.1722x; 1.1722x over previous
"""Masked multi-head attention (B=32, Lq=Lk=512, H=20, D=20) on 8 TRN2 NeuronCores.

Strategy (v2):
  - Work is decomposed into chunk-group jobs: (batch, kv-chunk pair, q-range).
    Since softmax numerator/denominator accumulate linearly over kv chunks and
    queries are independent, jobs of one batch can be split across rounds and
    merged on the host (device returns unnormalized O plus per-query sums).
  - Jobs are scheduled into SPMD rounds of 8 slots (one per core). Every round
    bakes (NQ, k chunks); q-ranges are split so pieces pack tightly into the
    baked widths (near-zero padding vs. the 1.6x loss of batch-atomic rounds).
  - Device per round: project Q (per head-group) and K/V (per chunk) from
    fp16 inputs with an appended ones-row (bias + kv masking + denominator
    column), then per (head-group, chunk): S^T = K_h @ Q_h^T (4 heads at
    32-partition offsets), P = exp(S/sqrt(D) - ESHIFT) on ACT (or DVE via an
    exp2 bit-trick for a tunable share), O^T += [V_h|1]^T @ P^T accumulated
    over the round's chunks. O^T (+ sums row) is copied to SBUF and DMA'd out
    unnormalized; the host merges partial jobs, divides by the sums, applies
    the query mask, and scatters into the final [32, 512, 400] output.
"""

import math

import numpy as np

import concourse.bacc as bacc
import concourse.bass as bass
import concourse.tile as tile
from concourse import mybir
from concourse.bass_utils import run_bass_kernel_spmd

B, LQ, LK = 32, 512, 512
H, D = 20, 20
OUT_DIM = H * D  # 400
N_CORES = 8
NG = 5   # head groups
HPG = 4  # heads per group (partition offsets 0/32/64/96)
KCH = 128
VW = H * 21 + 12  # 432: per-head 20 dims + ones col, padded for 32-wide slices
SCALE = 1.0 / math.sqrt(D)
ESHIFT = 6.0  # exp(s*SCALE - ESHIFT): softmax shift-invariant, keeps P in fp16

F32 = mybir.dt.float32
F16 = mybir.dt.float16
BF16 = mybir.dt.bfloat16
I16 = mybir.dt.int16

# Schraudolph exp2 bit trick (DVE path): bf16 bits of exp(x) ~ round(
#   128*log2(e)*x + (127*128 - C)).  C tunes the mantissa-linearization bias.
EXP_A = 128.0 * math.log2(math.e) * SCALE
EXP_C = 7.0
EXP_B = 16256.0 - 128.0 * math.log2(math.e) * ESHIFT - EXP_C

# Perf knobs
DVE_EXP_8 = 0   # of every 8 exp tiles, this many go to DVE (bit-trick)
TRACE = False
LAST_RESULT = None


# ----------------------------------------------------------------- planning

def _r16(x):
    return max(16, (int(x) + 15) // 16 * 16)


def _plan(q_len, v_len):
    """Decompose into jobs and schedule into SPMD rounds.

    Returns (rounds, assign) where rounds = [(NQ, k)] and
    assign[r][core] = (b, c0, q_lo, q_len_cols) or None."""
    nq = [_r16(min(int(q), LQ)) if int(q) > 0 else 16 for q in q_len]
    kv_eff = [LK if int(v) <= 0 else min(int(v), LK) for v in v_len]
    nkc = [math.ceil(k / KCH) for k in kv_eff]

    pairs, singles = [], []
    for b in range(B):
        k = nkc[b]
        for i in range(k // 2):
            pairs.append((nq[b], b, 2 * i))
        if k % 2:
            singles.append((nq[b], b, k - 1))

    def sched(jobs, k):
        """Split jobs at a cap, rank-match sorted pieces into rounds of 8.
        Search the cap for min predicted cost."""
        if not jobs:
            return [], []
        best = None
        for cap in (512, 448, 416, 384, 352, 320, 288, 256, 224):
            pieces = []
            for (n, b, c0) in jobs:
                q0 = 0
                while n - q0 > cap:
                    pieces.append((cap, b, c0, q0))
                    q0 += cap
                rem = n - q0
                if rem >= 16:
                    pieces.append((rem, b, c0, q0))
            pieces.sort(reverse=True)
            rounds, assign = [], []
            cost = 0.0
            for i in range(0, len(pieces), 8):
                grp = pieces[i:i + 8]
                NQ = grp[0][0]
                rounds.append((NQ, k))
                assign.append([(b, c0, q0, n) for (n, b, c0, q0) in grp]
                              + [None] * (8 - len(grp)))
                packs = 2 if NQ > 256 else 1
                pe = 40 * k * NQ + 5 * NQ + 640 * k + 432 * k
                act = 5 * k * (4 * NQ + 222 * packs)
                dve = 10 * NQ + 640 * k + 432 * k + 900 * k
                cost += max(pe / 2.4, act / 1.2, dve / 0.96) + 400
            if best is None or cost < best[0]:
                best = (cost, rounds, assign)
        return best[1], best[2]

    r2, a2 = sched(pairs, 2)
    r1, a1 = sched(singles, 1)
    rounds = r2 + r1
    assign = a2 + a1
    return rounds, assign


# ------------------------------------------------------------ host packing

def _pack_qk_weights(W, bias):
    """[400, 20] linear weight -> [21, NG*128] lhsT (head 4g+j at columns
    128g+32j..+20; row 20 = bias)."""
    t = np.zeros((D + 1, NG * 128), np.float16)
    for h in range(H):
        g, j = divmod(h, HPG)
        c = g * 128 + 32 * j
        t[:D, c:c + D] = W[h * D:(h + 1) * D, :].T.astype(np.float16)
        t[D, c:c + D] = bias[h * D:(h + 1) * D].astype(np.float16)
    return t


def _pack_v_weights(W, bias):
    """[400, 20] -> [21, 432] rhs: head h at cols 21h..21h+19, ones col at
    21h+20."""
    t = np.zeros((D + 1, VW), np.float16)
    for h in range(H):
        c = 21 * h
        t[:D, c:c + D] = W[h * D:(h + 1) * D, :].T.astype(np.float16)
        t[D, c:c + D] = bias[h * D:(h + 1) * D].astype(np.float16)
        t[D, c + D] = 1.0
    return t


# ------------------------------------------------------------ device build

def _emit(tc, nc, dr, rounds):
    with (
        tc.tile_pool(name="wpool", bufs=1) as wpool,
        tc.tile_pool(name="seqin", bufs=2) as seqp,
        tc.tile_pool(name="sbq", bufs=2) as sbqp,
        tc.tile_pool(name="sbk", bufs=2) as sbkp,
        tc.tile_pool(name="sbv", bufs=3) as sbvp,
        tc.tile_pool(name="sbp", bufs=4) as sbpp,
        tc.tile_pool(name="sbo", bufs=3) as sbop,
        tc.tile_pool(name="ppj", bufs=2, space="PSUM") as ppj,
        tc.tile_pool(name="pss", bufs=2, space="PSUM") as pss,
        tc.tile_pool(name="pso", bufs=2, space="PSUM") as pso,
    ):
        wq = wpool.tile([D + 1, NG * 128], F16, tag="wq")
        nc.sync.dma_start(wq[:], dr["wq"])
        wk = wpool.tile([D + 1, NG * 128], F16, tag="wk")
        nc.sync.dma_start(wk[:], dr["wk"])
        wv = wpool.tile([D + 1, VW], F16, tag="wv")
        nc.sync.dma_start(wv[:], dr["wv"])
        eshift = wpool.tile([128, 1], F32, tag="eshift")
        nc.vector.memset(eshift[:], -ESHIFT)

        exp_i = 0
        for r, (NQ, k) in enumerate(rounds):
            nkv = k * KCH
            qt = seqp.tile([D + 1, NQ], F16, tag="qt", name=f"qt{r}")
            nc.sync.dma_start(qt[:], dr[f"qt{r}"])
            kt = seqp.tile([D + 1, nkv], F16, tag="kt", name=f"kt{r}")
            nc.sync.dma_start(kt[:], dr[f"kt{r}"])
            vt = seqp.tile([D + 1, nkv], F16, tag="vt", name=f"vt{r}")
            nc.sync.dma_start(vt[:], dr[f"vt{r}"])

            # projections
            q = sbqp.tile([128, NG, NQ], F16, tag="q", name=f"q{r}")
            for g in range(NG):
                pq = ppj.tile([128, 512], F32, tag="ppj")
                nc.tensor.matmul(pq[:, :NQ], wq[:, g * 128:(g + 1) * 128],
                                 qt[:], start=True, stop=True)
                nc.vector.tensor_copy(q[:, g, :], pq[:, :NQ])
            kk = sbkp.tile([128, NG, nkv], F16, tag="k", name=f"k{r}")
            for g in range(NG):
                pk = ppj.tile([128, 512], F32, tag="ppj")
                nc.tensor.matmul(pk[:, :nkv], wk[:, g * 128:(g + 1) * 128],
                                 kt[:], start=True, stop=True)
                nc.vector.tensor_copy(kk[:, g, :], pk[:, :nkv])
            sbV = []
            for c in range(k):
                pv = ppj.tile([128, 512], F32, tag="ppj")
                nc.tensor.matmul(pv[:, :VW], vt[:, c * KCH:(c + 1) * KCH],
                                 wv[:], start=True, stop=True)
                v = sbvp.tile([128, VW], F16, tag="v", name=f"v{r}_{c}")
                nc.vector.tensor_copy(v[:], pv[:, :VW])
                sbV.append(v)

            packs, hp = 2, 2
            for g in range(NG):
                po = pso.tile([128, NQ], F32, tag="po", name=f"po{r}_{g}")
                for c in range(k):
                    packtiles = []
                    for ip in range(packs):
                        ps = pss.tile([128, hp, 512], F32, tag="pss",
                                      name=f"ps{r}_{g}_{c}_{ip}")
                        for jj in range(hp):
                            j = ip * hp + jj
                            nc.tensor.matmul(
                                ps[:, jj, :NQ],
                                kk[32 * j:32 * j + D, g, c * KCH:(c + 1) * KCH],
                                q[32 * j:32 * j + D, g, :],
                                start=True, stop=True,
                                tile_position=(32 * j, 0),
                            )
                        packtiles.append(ps)
                    ptiles = []
                    for ip, ps in enumerate(packtiles):
                        if (exp_i % 8) < DVE_EXP_8:
                            p = sbpp.tile([128, hp, NQ], I16, tag="sbp",
                                          name=f"p{r}_{g}_{c}_{ip}")
                            nc.vector.tensor_scalar(
                                p[:], ps[:, :, :NQ],
                                EXP_A, EXP_B,
                                mybir.AluOpType.mult, mybir.AluOpType.add,
                            )
                            ptiles.append((p.bitcast(BF16), ip))
                        else:
                            p = sbpp.tile([128, hp, NQ], F16, tag="sbp",
                                          name=f"p{r}_{g}_{c}_{ip}")
                            nc.scalar.activation(
                                p[:], ps[:, :, :NQ],
                                mybir.ActivationFunctionType.Exp,
                                bias=eshift[:], scale=SCALE,
                            )
                            ptiles.append((p, ip))
                        exp_i += 1
                    for p, ip in ptiles:
                        for jj in range(hp):
                            j = ip * hp + jj
                            h = HPG * g + j
                            nc.tensor.matmul(
                                po[32 * j:32 * j + 32, :],
                                sbV[c][:, 21 * h:21 * h + 32],
                                p[:, jj, :],
                                start=(c == 0), stop=(c == k - 1),
                                tile_position=(0, 32 * j),
                                skip_group_check=True,
                            )
                o = sbop.tile([128, NQ], F32, tag="o", name=f"o{r}_{g}")
                nc.vector.tensor_copy(o[:], po[:])
                nc.sync.dma_start(dr[f"o{r}"][g], o[:])


def _build_nc(rounds):
    nc = bacc.Bacc(
        "TRN2",
        target_bir_lowering=False,
        debug=False,
        enable_asserts=False,
        num_devices=N_CORES,
    )
    dr = {}
    for r, (NQ, k) in enumerate(rounds):
        nkv = k * KCH
        dr[f"qt{r}"] = nc.dram_tensor(f"qt{r}", [D + 1, NQ], F16, kind="ExternalInput").ap()
        dr[f"kt{r}"] = nc.dram_tensor(f"kt{r}", [D + 1, nkv], F16, kind="ExternalInput").ap()
        dr[f"vt{r}"] = nc.dram_tensor(f"vt{r}", [D + 1, nkv], F16, kind="ExternalInput").ap()
        dr[f"o{r}"] = nc.dram_tensor(f"o{r}", [NG, 128, NQ], F32, kind="ExternalOutput").ap()
    dr["wq"] = nc.dram_tensor("wq", [D + 1, NG * 128], F16, kind="ExternalInput").ap()
    dr["wk"] = nc.dram_tensor("wk", [D + 1, NG * 128], F16, kind="ExternalInput").ap()
    dr["wv"] = nc.dram_tensor("wv", [D + 1, VW], F16, kind="ExternalInput").ap()

    with tile.TileContext(nc) as tc:
        _emit(tc, nc, dr, rounds)
    nc.compile()
    return nc


# ----------------------------------------------------------------- driver

def kernel(**inputs):
    global LAST_RESULT
    Q_seq = np.asarray(inputs["Q_seq"], dtype=np.float32)
    K_seq = np.asarray(inputs["K_seq"], dtype=np.float32)
    V_seq = np.asarray(inputs["V_seq"], dtype=np.float32)
    Q_len = np.asarray(inputs["Q_len"]).reshape(-1).astype(np.int64)
    V_len = np.asarray(inputs["V_len"]).reshape(-1).astype(np.int64)

    rounds, assign = _plan(Q_len, V_len)
    nc = _build_nc(rounds)

    wq = _pack_qk_weights(np.asarray(inputs["WQ_w"], np.float32),
                          np.asarray(inputs["WQ_b"], np.float32))
    wk = _pack_qk_weights(np.asarray(inputs["WK_w"], np.float32),
                          np.asarray(inputs["WK_b"], np.float32))
    wv = _pack_v_weights(np.asarray(inputs["WV_w"], np.float32),
                         np.asarray(inputs["WV_b"], np.float32))

    Qt = np.ascontiguousarray(Q_seq.transpose(0, 2, 1)).astype(np.float16)
    Kt = np.ascontiguousarray(K_seq.transpose(0, 2, 1)).astype(np.float16)
    Vt = np.ascontiguousarray(V_seq.transpose(0, 2, 1)).astype(np.float16)

    in_maps = []
    for core in range(N_CORES):
        m = {"wq": wq, "wk": wk, "wv": wv}
        for r, (NQ, k) in enumerate(rounds):
            nkv = k * KCH
            qt = np.zeros((D + 1, NQ), np.float16)
            kt = np.zeros((D + 1, nkv), np.float16)
            vt = np.zeros((D + 1, nkv), np.float16)
            job = assign[r][core]
            if job is not None:
                b, c0, q0, qn = job
                qt[:D, :qn] = Qt[b][:, q0:q0 + qn]
                qt[D, :qn] = 1.0
                kv_eff = LK if int(V_len[b]) <= 0 else min(int(V_len[b]), LK)
                lo = c0 * KCH
                n = max(0, min(kv_eff - lo, nkv))
                if n > 0:
                    kt[:D, :n] = Kt[b][:, lo:lo + n]
                    kt[D, :n] = 1.0
                    vt[:D, :n] = Vt[b][:, lo:lo + n]
                    vt[D, :n] = 1.0
            m[f"qt{r}"] = qt
            m[f"kt{r}"] = kt
            m[f"vt{r}"] = vt
        in_maps.append(m)

    res = run_bass_kernel_spmd(
        nc, in_maps, core_ids=list(range(N_CORES)), trace=TRACE
    )
    LAST_RESULT = res

    num = {}
    den = {}
    for core in range(N_CORES):
        for r, (NQ, k) in enumerate(rounds):
            job = assign[r][core]
            if job is None:
                continue
            b, c0, q0, qn = job
            o = res.results[core][f"o{r}"]  # [NG, 128, NQ]
            if b not in num:
                nqb = _r16(min(int(Q_len[b]), LQ)) if int(Q_len[b]) > 0 else 16
                num[b] = np.zeros((H, D, nqb), np.float32)
                den[b] = np.zeros((H, nqb), np.float32)
            oo = o.reshape(NG, HPG, 32, NQ)[:, :, :21, :].reshape(H, 21, NQ)
            num[b][:, :, q0:q0 + qn] += oo[:, :D, :qn]
            den[b][:, q0:q0 + qn] += oo[:, D, :qn]

    out = np.zeros((B, LQ, OUT_DIM), np.float32)
    for b in range(B):
        ql = int(Q_len[b])
        if ql <= 0 or b not in num:
            continue
        ql = min(ql, LQ)
        o = num[b][:, :, :ql] / den[b][:, None, :ql]  # [H, D, ql]
        out[b, :ql, :] = o.transpose(2, 0, 1).reshape(ql, OUT_DIM)
    return out


# revision 13
# speedup vs baseline: 1.9787x; 1.6879x over previous
"""Masked multi-head attention (B=32, Lq=Lk=512, H=20, D=20) on 8 TRN2 NeuronCores.

Strategy (v3):
  - Work decomposes into chunk-group jobs (batch, kv-chunk pair, q-range):
    softmax numerator/denominator accumulate linearly over kv chunks and
    queries are independent, so jobs split freely across SPMD rounds; the
    host merges partial (O, sum) outputs and normalizes.
  - Jobs are scheduled into rounds of 8 slots (one per core) with baked
    (NQ, k); q-splitting packs pieces tightly into the baked widths.
  - The host pre-projects Q/K/V (tiny [400,20] weights) straight into the
    PE-ready SBUF layouts (heads at 32-partition offsets for Q/K; per-head
    21-column blocks with a masked ones-column for V, which yields the
    softmax denominator as a free output row).  The device does only the
    quadratic work: per (head-group, chunk) S^T = K_h @ Q_h^T (4 heads at
    32-partition row offsets of the PE), P = exp(S/sqrt(D) - ESHIFT) on the
    ACT engine or on DVE via a Schraudolph exp2 bit-trick (tunable split,
    balancing the two PSUM-draining engines), then O^T += [V_h|1]^T @ P^T
    accumulated over the round's chunks in PSUM.
  - Each head's S^T slice owns full 2KB PSUM banks ([128, 2, 512] tile per
    2 heads): hardware PSUM accumulation groups are bank-granular.
"""

import math

import numpy as np

import concourse.bacc as bacc
import concourse.bass as bass
import concourse.tile as tile
from concourse import mybir
from concourse.bass_utils import run_bass_kernel_spmd

B, LQ, LK = 32, 512, 512
H, D = 20, 20
OUT_DIM = H * D  # 400
N_CORES = 8
NG = 5   # head groups
HPG = 4  # heads per group (partition offsets 0/32/64/96)
KCH = 128
VW = H * 21 + 12  # 432: per-head 20 dims + ones col, padded for 32-wide slices
SCALE = 1.0 / math.sqrt(D)
ESHIFT = 6.0  # exp(s*SCALE - ESHIFT): softmax shift-invariant, keeps P in fp16

F32 = mybir.dt.float32
F16 = mybir.dt.float16
BF16 = mybir.dt.bfloat16
I16 = mybir.dt.int16

# Schraudolph exp2 bit trick (DVE path): bf16 bits of exp(x) ~ round(
#   128*log2(e)*x + (127*128 - C)).  C tunes the mantissa-linearization bias.
EXP_A = 128.0 * math.log2(math.e) * SCALE
EXP_C = 5.0
EXP_B = 16256.0 - 128.0 * math.log2(math.e) * ESHIFT - EXP_C

# Perf knobs
DVE_EXP_8 = 2   # of every 8 exp tiles, this many go to DVE (bit-trick)
TRACE = False
LAST_RESULT = None


# ----------------------------------------------------------------- planning

def _r16(x):
    return max(16, (int(x) + 15) // 16 * 16)


def _plan(q_len, v_len):
    """Decompose into jobs and schedule into SPMD rounds.

    Returns (rounds, assign) where rounds = [(NQ, k)] and
    assign[r][core] = (b, c0, q_lo, q_len_cols) or None."""
    nq = [_r16(min(int(q), LQ)) if int(q) > 0 else 16 for q in q_len]
    kv_eff = [LK if int(v) <= 0 else min(int(v), LK) for v in v_len]
    nkc = [math.ceil(k / KCH) for k in kv_eff]

    pairs, singles = [], []
    for b in range(B):
        k = nkc[b]
        for i in range(k // 2):
            pairs.append((nq[b], b, 2 * i))
        if k % 2:
            singles.append((nq[b], b, k - 1))

    def sched(jobs, k):
        """Split jobs at a cap, rank-match sorted pieces into rounds of 8.
        Search the cap for min predicted cost."""
        if not jobs:
            return [], []
        best = None
        for cap in (512, 448, 384, 320, 288, 256, 224, 192):
            pieces = []
            for (n, b, c0) in jobs:
                q0 = 0
                while n - q0 > cap:
                    pieces.append((cap, b, c0, q0))
                    q0 += cap
                rem = n - q0
                if rem >= 16:
                    pieces.append((rem, b, c0, q0))
            pieces.sort(reverse=True)
            rounds, assign = [], []
            cost = 0.0
            for i in range(0, len(pieces), 8):
                grp = pieces[i:i + 8]
                NQ = grp[0][0]
                rounds.append((NQ, k))
                assign.append([(b, c0, q0, n) for (n, b, c0, q0) in grp]
                              + [None] * (8 - len(grp)))
                # engine-time model (ns): PE streams, ACT/DVE drain pool
                pe = (40 * k * NQ) / 2.4
                drain = (20 * k * NQ + 5 * NQ          # exp + O copy
                         + 10 * k * 280 + 5 * 230) / 2.16
                cost += max(pe, drain) + 600
            if best is None or cost < best[0]:
                best = (cost, rounds, assign)
        return best[1], best[2]

    r2, a2 = sched(pairs, 2)
    r1, a1 = sched(singles, 1)
    rounds = r2 + r1
    assign = a2 + a1
    return rounds, assign


# ------------------------------------------------------------ device build

def _emit(tc, nc, dr, rounds):
    with (
        tc.tile_pool(name="wpool", bufs=1) as wpool,
        tc.tile_pool(name="seqin", bufs=2) as seqp,
        tc.tile_pool(name="sbp", bufs=4) as sbpp,
        tc.tile_pool(name="sbo", bufs=3) as sbop,
        tc.tile_pool(name="pss", bufs=3, space="PSUM") as pss,
        tc.tile_pool(name="pso", bufs=2, space="PSUM") as pso,
    ):
        eshift = wpool.tile([128, 1], F32, tag="eshift")
        nc.vector.memset(eshift[:], -ESHIFT)

        exp_i = 0
        for r, (NQ, k) in enumerate(rounds):
            nkv = k * KCH
            qp = seqp.tile([128, NG, NQ], F16, tag="qp", name=f"qp{r}")
            nc.sync.dma_start(qp[:], dr[f"qp{r}"])
            kp = seqp.tile([128, NG, nkv], F16, tag="kp", name=f"kp{r}")
            nc.sync.dma_start(kp[:], dr[f"kp{r}"])
            vp = seqp.tile([128, k, VW], F16, tag="vp", name=f"vp{r}")
            nc.sync.dma_start(vp[:], dr[f"vp{r}"])

            o = sbop.tile([128, NG, NQ], F32, tag="o", name=f"o{r}")
            for g in range(NG):
                po = pso.tile([128, NQ], F32, tag="po", name=f"po{r}_{g}")
                for c in range(k):
                    pp = []
                    for ip in range(2):
                        ps = pss.tile([128, 2, 512], F32, tag="pss",
                                      name=f"ps{r}_{g}_{c}_{ip}")
                        for jj in range(2):
                            j = 2 * ip + jj
                            nc.tensor.matmul(
                                ps[:, jj, :NQ],
                                kp[32 * j:32 * j + D, g,
                                   c * KCH:(c + 1) * KCH],
                                qp[32 * j:32 * j + D, g, :],
                                start=True, stop=True,
                                tile_position=(32 * j, 0),
                            )
                        pp.append(ps)
                    for ip in range(2):
                        if (exp_i % 8) < DVE_EXP_8:
                            p = sbpp.tile([128, 2, NQ], I16, tag="sbp",
                                          name=f"p{r}_{g}_{c}_{ip}")
                            nc.vector.tensor_scalar(
                                p[:], pp[ip][:, :, :NQ],
                                EXP_A, EXP_B,
                                mybir.AluOpType.mult, mybir.AluOpType.add,
                            )
                            p = p.bitcast(BF16)
                        else:
                            p = sbpp.tile([128, 2, NQ], F16, tag="sbp",
                                          name=f"p{r}_{g}_{c}_{ip}")
                            nc.scalar.activation(
                                p[:], pp[ip][:, :, :NQ],
                                mybir.ActivationFunctionType.Exp,
                                bias=eshift[:], scale=SCALE,
                            )
                        exp_i += 1
                        for jj in range(2):
                            j = 2 * ip + jj
                            h = HPG * g + j
                            nc.tensor.matmul(
                                po[32 * j:32 * j + 32, :],
                                vp[:, c, 21 * h:21 * h + 32],
                                p[:, jj, :],
                                start=(c == 0), stop=(c == k - 1),
                                tile_position=(0, 32 * j),
                                skip_group_check=True,
                            )
                nc.vector.tensor_copy(o[:, g, :], po[:])
            nc.sync.dma_start(dr[f"o{r}"], o[:])


def _build_nc(rounds):
    nc = bacc.Bacc(
        "TRN2",
        target_bir_lowering=False,
        debug=False,
        enable_asserts=False,
        num_devices=N_CORES,
    )
    dr = {}
    for r, (NQ, k) in enumerate(rounds):
        nkv = k * KCH
        dr[f"qp{r}"] = nc.dram_tensor(f"qp{r}", [128, NG, NQ], F16, kind="ExternalInput").ap()
        dr[f"kp{r}"] = nc.dram_tensor(f"kp{r}", [128, NG, nkv], F16, kind="ExternalInput").ap()
        dr[f"vp{r}"] = nc.dram_tensor(f"vp{r}", [128, k, VW], F16, kind="ExternalInput").ap()
        dr[f"o{r}"] = nc.dram_tensor(f"o{r}", [128, NG, NQ], F32, kind="ExternalOutput").ap()

    with tile.TileContext(nc) as tc:
        _emit(tc, nc, dr, rounds)
    nc.compile()
    return nc


# ----------------------------------------------------------------- driver

def _project(seq, W, bias):
    """[B, L, 20] @ [400, 20].T + b -> [B, L, 400] fp32."""
    x = seq.reshape(-1, D).astype(np.float32)
    return (x @ W.astype(np.float32).T + bias.astype(np.float32)).reshape(
        seq.shape[0], seq.shape[1], OUT_DIM)


def _prep_core_inputs(core, rounds, assign, V_len, QL, KL, VP):
    m = {}
    for r, (NQ, k) in enumerate(rounds):
        nkv = k * KCH
        qp = np.zeros((128, NG, NQ), np.float16)
        kp = np.zeros((128, NG, nkv), np.float16)
        vp = np.zeros((128, k, VW), np.float16)
        job = assign[r][core]
        if job is not None:
            b, c0, q0, qn = job
            qp.reshape(HPG, 32, NG, NQ)[:, :D, :, :qn] = \
                QL[b][:, :, :, q0:q0 + qn]
            kv_eff = LK if int(V_len[b]) <= 0 else min(int(V_len[b]), LK)
            lo = c0 * KCH
            n = max(0, min(kv_eff - lo, nkv))
            if n > 0:
                kp.reshape(HPG, 32, NG, nkv)[:, :D, :, :n] = \
                    KL[b][:, :, :, lo:lo + n]
                vblock = VP[b][lo:lo + n].reshape(n, H, D)
                for c in range((n + KCH - 1) // KCH):
                    nn = min(KCH, n - c * KCH)
                    tmp = np.zeros((nn, H, 21), np.float16)
                    tmp[:, :, :D] = vblock[c * KCH:c * KCH + nn]
                    tmp[:, :, D] = 1.0
                    vp[:nn, c, :H * 21] = tmp.reshape(nn, H * 21)
        m[f"qp{r}"] = qp
        m[f"kp{r}"] = kp
        m[f"vp{r}"] = vp
    return m


def kernel(**inputs):
    global LAST_RESULT
    Q_seq = np.asarray(inputs["Q_seq"], dtype=np.float32)
    K_seq = np.asarray(inputs["K_seq"], dtype=np.float32)
    V_seq = np.asarray(inputs["V_seq"], dtype=np.float32)
    Q_len = np.asarray(inputs["Q_len"]).reshape(-1).astype(np.int64)
    V_len = np.asarray(inputs["V_len"]).reshape(-1).astype(np.int64)

    rounds, assign = _plan(Q_len, V_len)
    nc = _build_nc(rounds)

    # host projections into PE layouts
    QP = _project(Q_seq, np.asarray(inputs["WQ_w"]), np.asarray(inputs["WQ_b"]))
    KP = _project(K_seq, np.asarray(inputs["WK_w"]), np.asarray(inputs["WK_b"]))
    VP = _project(V_seq, np.asarray(inputs["WV_w"]), np.asarray(inputs["WV_b"]))
    # q/k layout: [b, L, H=(g,j), D] -> [b][j, d, g, col]
    QL = QP.reshape(B, LQ, NG, HPG, D).transpose(0, 3, 4, 2, 1).astype(np.float16)
    KL = KP.reshape(B, LK, NG, HPG, D).transpose(0, 3, 4, 2, 1).astype(np.float16)
    VP = VP.astype(np.float16)

    in_maps = [
        _prep_core_inputs(core, rounds, assign, V_len, QL, KL, VP)
        for core in range(N_CORES)
    ]

    res = run_bass_kernel_spmd(
        nc, in_maps, core_ids=list(range(N_CORES)), trace=TRACE
    )
    LAST_RESULT = res

    num = {}
    den = {}
    for core in range(N_CORES):
        for r, (NQ, k) in enumerate(rounds):
            job = assign[r][core]
            if job is None:
                continue
            b, c0, q0, qn = job
            o = res.results[core][f"o{r}"]  # [128, NG, NQ]
            if b not in num:
                nqb = _r16(min(int(Q_len[b]), LQ)) if int(Q_len[b]) > 0 else 16
                num[b] = np.zeros((H, D, nqb), np.float32)
                den[b] = np.zeros((H, nqb), np.float32)
            oo = o.reshape(HPG, 32, NG, NQ).transpose(2, 0, 1, 3)  # [g,j,32,q]
            oo = oo.reshape(H, 32, NQ)
            num[b][:, :, q0:q0 + qn] += oo[:, :D, :qn]
            den[b][:, q0:q0 + qn] += oo[:, D, :qn]

    out = np.zeros((B, LQ, OUT_DIM), np.float32)
    for b in range(B):
        ql = int(Q_len[b])
        if ql <= 0 or b not in num:
            continue
        ql = min(ql, LQ)
        o = num[b][:, :, :ql] / den[b][:, None, :ql]  # [H, D, ql]
        out[b, :ql, :] = o.transpose(2, 0, 1).reshape(ql, OUT_DIM)
    return out
